# revision 6
# baseline (speedup 1.0000x reference)
"""Trainium2 kernel for nn_CDR_75642964017548.

Computes, for x[B=1024, D=1024] and basis[O=256, D=1024]:
    d1[b,o] = sum_d |x[b,d] - basis[o,d]|           (L1, temperature 1.0)
    d2[b,o] = sqrt(sum_d (x[b,d] - basis[o,d])^2)   (L2, temperature 2.0)
    xd = d1 + 0.5*d2
    out[b,o] = -(xd*(1+ALPHA) - ALPHA*sum_o' xd[b,o'])

Sharding: output/centroid-parallel. Each of the 8 cores gets 32 basis rows
and the full x (replicated). Device computes xd[32, 1024] per core; host
gathers [256, 1024], applies the (tiny) alpha rowsum correction and
transposes to [1024, 256].

Device layout: D on partitions (8 chunks of 128), B on the free dim.

L1 via the min-trick (TensorScalar has no abs op on TRN2):
    sum_d |t| = sum_d t - 2*sum_d min(t, 0),   t = x - c
    sum_d t   = sx[b] - sc[o]    (sx = sum_d x, sc = sum_d c, host-prepped)
so the only per-(o,chunk) DVE work is one fp16 tensor_scalar
(op0=subtract with the per-partition basis column, op1=min vs 0.0), and
the partition-reduction runs on TensorE with "-2 selector" weights
(columns of -2 at position o) accumulating all 32 centroids and 8 chunks
into one [32, B] PSUM group. A K=1 ones-matmul adds the sx row.

L2: ||x-c||^2 = ||x||^2 + ||c||^2 - 2*x.c via PE matmul of (-2*basis)^T
against x chunks plus a K=1 ones-matmul adding ||x||^2; one ScalarE
activation computes sqrt(0.25*psum + 0.25*||c||^2) = 0.5*d2.

Finalize: one scalar_tensor_tensor: xd = (d1_psum + (-sc)) + 0.5*d2.
"""

import numpy as np

B, O, D = 1024, 256, 1024
NCORES = 8
OSH = O // NCORES          # 32 centroids per core
NCHUNK = D // 128          # 8 partition chunks
ALPHA = 0.005

_cache = {}


def _build():
    import concourse.bass as bass
    import concourse.bacc as bacc
    import concourse.tile as tile
    from concourse import mybir

    f32 = mybir.dt.float32
    f16 = mybir.dt.float16
    Alu = mybir.AluOpType
    Act = mybir.ActivationFunctionType

    nc = bacc.Bacc(
        "TRN2",
        target_bir_lowering=False,
        debug=False,
        enable_asserts=False,
        num_devices=NCORES,
    )

    # DRAM I/O (flat free-dim layouts; column index = chunk*width + inner)
    xT_d = nc.dram_tensor("xT", [128, NCHUNK * B], f16, kind="ExternalInput").ap()
    bT_d = nc.dram_tensor("bT", [128, NCHUNK * OSH], f32, kind="ExternalInput").ap()
    bm2_d = nc.dram_tensor("bm2", [128, NCHUNK * OSH], f16, kind="ExternalInput").ap()
    xsq_d = nc.dram_tensor("xsq", [1, B], f16, kind="ExternalInput").ap()
    sx_d = nc.dram_tensor("sx", [1, B], f16, kind="ExternalInput").ap()
    csq_d = nc.dram_tensor("csq", [OSH, 1], f32, kind="ExternalInput").ap()
    msc_d = nc.dram_tensor("msc", [OSH, 1], f32, kind="ExternalInput").ap()
    sel_d = nc.dram_tensor("sel", [128, OSH * OSH], f16, kind="ExternalInput").ap()
    out_d = nc.dram_tensor("xd", [OSH, B], f32, kind="ExternalOutput").ap()

    NJ = B // 512

    with tile.TileContext(nc) as tc:
        with (
            tc.tile_pool(name="const", bufs=1) as const,
            tc.tile_pool(name="absp", bufs=6) as absp,
            tc.tile_pool(name="fin", bufs=1) as fin,
            tc.tile_pool(name="psum", bufs=1, space="PSUM") as psum,
        ):
            # x chunks as 8 separate tiles so compute can start per-chunk
            xTc = []
            for c in range(NCHUNK):
                t = const.tile([128, B], f16, tag=f"xT{c}")
                nc.sync.dma_start(t[:], xT_d[:, c * B : (c + 1) * B])
                xTc.append(t)
            bT = const.tile([128, NCHUNK * OSH], f32, tag="bT")
            nc.sync.dma_start(bT[:], bT_d[:])
            bm2 = const.tile([128, NCHUNK * OSH], f16, tag="bm2")
            nc.sync.dma_start(bm2[:], bm2_d[:])
            xsq = const.tile([1, B], f16, tag="xsq")
            nc.sync.dma_start(xsq[:], xsq_d[:])
            sx = const.tile([1, B], f16, tag="sx")
            nc.sync.dma_start(sx[:], sx_d[:])
            csq = const.tile([OSH, 1], f32, tag="csq")
            nc.sync.dma_start(csq[:], csq_d[:])
            msc = const.tile([OSH, 1], f32, tag="msc")
            nc.sync.dma_start(msc[:], msc_d[:])
            sel = const.tile([128, OSH * OSH], f16, tag="sel")
            nc.sync.dma_start(sel[:], sel_d[:])

            ones32 = const.tile([1, OSH], f16, tag="ones32")
            nc.vector.memset(ones32[:], 1.0)

            xc_ps = psum.tile([OSH, B], f32, tag="xc")
            d1_ps = psum.tile([OSH, B], f32, tag="d1")

            # ---- L2 part: xc_ps = -2*x.c + ||x||^2 ----
            for j in range(NJ):
                sl = slice(j * 512, (j + 1) * 512)
                for c in range(NCHUNK):
                    nc.tensor.matmul(
                        xc_ps[:, sl],
                        bm2[:, c * OSH : (c + 1) * OSH],
                        xTc[c][:, sl],
                        start=(c == 0),
                        stop=False,
                    )
                nc.tensor.matmul(
                    xc_ps[:, sl], ones32[:], xsq[:, sl], start=False, stop=True
                )

            # ---- L1 part: d1_ps = -2 * sum_d min(x-c, 0) + sx ----
            for o in range(OSH):
                for c in range(NCHUNK):
                    a = absp.tile([128, B], f16, tag="abs")
                    nc.vector.tensor_scalar(
                        out=a[:],
                        in0=xTc[c][:],
                        scalar1=bT[:, c * OSH + o : c * OSH + o + 1],
                        scalar2=0.0,
                        op0=Alu.subtract,
                        op1=Alu.min,
                    )
                    for j in range(NJ):
                        sl = slice(j * 512, (j + 1) * 512)
                        nc.tensor.matmul(
                            d1_ps[:, sl],
                            sel[:, o * OSH : (o + 1) * OSH],
                            a[:, sl],
                            start=(o == 0 and c == 0),
                            stop=False,
                        )
            for j in range(NJ):
                sl = slice(j * 512, (j + 1) * 512)
                nc.tensor.matmul(
                    d1_ps[:, sl], ones32[:], sx[:, sl], start=False, stop=True
                )

            # ---- finalize: xd = (d1_ps - sc) + sqrt(0.25*xc_ps + 0.25*csq) ----
            h2 = fin.tile([OSH, B], f32, tag="h2")
            nc.scalar.activation(h2[:], xc_ps[:], Act.Sqrt, bias=csq[:], scale=0.25)
            xd = fin.tile([OSH, B], f32, tag="xd")
            nc.vector.scalar_tensor_tensor(
                out=xd[:],
                in0=d1_ps[:],
                scalar=msc[:],
                in1=h2[:],
                op0=Alu.add,
                op1=Alu.add,
            )
            nc.sync.dma_start(out_d[:], xd[:])

    nc.compile()
    return nc


def _sel_matrix():
    if "sel" not in _cache:
        sel = np.zeros((128, OSH, OSH), dtype=np.float16)
        for o in range(OSH):
            sel[:, o, o] = -2.0
        _cache["sel"] = np.ascontiguousarray(sel.reshape(128, OSH * OSH))
    return _cache["sel"]


def _prep_inputs(x: np.ndarray, basis: np.ndarray):
    """Build the 8 per-core input maps (host-side shard + layout prep)."""
    xT = np.ascontiguousarray(x.T)  # [D, B] f32
    xT16 = (
        xT.astype(np.float16)
        .reshape(NCHUNK, 128, B)
        .transpose(1, 0, 2)
        .reshape(128, NCHUNK * B)
    )
    xT16 = np.ascontiguousarray(xT16)
    xsq16 = (x * x).sum(axis=1, dtype=np.float32).astype(np.float16)[None, :]
    sx16 = x.sum(axis=1, dtype=np.float32).astype(np.float16)[None, :]

    in_maps = []
    for k in range(NCORES):
        bs = basis[k * OSH : (k + 1) * OSH]  # [32, D] f32
        bT = (
            np.ascontiguousarray(bs.T)
            .reshape(NCHUNK, 128, OSH)
            .transpose(1, 0, 2)
            .reshape(128, NCHUNK * OSH)
        )
        bT = np.ascontiguousarray(bT).astype(np.float32)
        bm2 = np.ascontiguousarray((-2.0 * bT).astype(np.float16))
        csq = (0.25 * (bs * bs).sum(axis=1, dtype=np.float32)).astype(np.float32)[
            :, None
        ]
        msc = (-bs.sum(axis=1, dtype=np.float32)).astype(np.float32)[:, None]
        in_maps.append(
            {
                "xT": xT16,
                "bT": bT,
                "bm2": bm2,
                "xsq": xsq16,
                "sx": sx16,
                "csq": np.ascontiguousarray(csq),
                "msc": np.ascontiguousarray(msc),
                "sel": _sel_matrix(),
            }
        )
    return in_maps


def _run(x: np.ndarray, basis: np.ndarray, trace: bool = False):
    from concourse import bass_utils

    if "nc" not in _cache:
        _cache["nc"] = _build()
    nc = _cache["nc"]
    in_maps = _prep_inputs(x, basis)
    res = bass_utils.run_bass_kernel_spmd(
        nc, in_maps, core_ids=list(range(NCORES)), trace=trace
    )
    return res


def _postprocess(xd_parts) -> np.ndarray:
    xd = np.concatenate(xd_parts, axis=0)  # [O, B] f32
    s = xd.sum(axis=0, dtype=np.float32)  # [B]
    out = ALPHA * s[:, None] - (1.0 + ALPHA) * xd.T  # [B, O]
    return np.ascontiguousarray(out.astype(np.float32))


def kernel(x: np.ndarray, basis: np.ndarray) -> np.ndarray:
    res = _run(x, basis, trace=False)
    return _postprocess([r["xd"] for r in res.results])
